# revision 2
# baseline (speedup 1.0000x reference)
"""Allegro-style equivariant GNN edge-network on 8 TRN2 NeuronCores — v2.

Data-parallel over edges (EC=16384/core), 512-edge blocks (32/core).
vs baseline v1 (753us):
  - 1-bank psum tiles (N=512) + deep pools -> cross-block pipelining keeps the
    PE dense (HAM stays at 2.4GHz; v1 ran 94% of the span at 1.2GHz)
  - big fused DVE muls via stride-0 free-dim repeat (x2 rep9 / x1 rep4)
  - DVE reads PSUM operands directly (env via PE-replicated Wenv128 weights)
  - biases folded into weights via ones rows / ACT bias (no Identity+bias ops)
  - t000 env-path folded into a comb0-side matmul term (WG01)
"""

import sys

sys.path.insert(0, "/opt/trn_rl_repo")

import numpy as np
import ml_dtypes

BF = ml_dtypes.bfloat16

import concourse.bass as bass
import concourse.mybir as mybir
from concourse import bacc
from concourse.tile import TileContext
from concourse.bass_utils import run_bass_kernel_spmd

E = 131072
NCORES = 8
EC = E // NCORES
C = 16
S = 64
NB = 8
TE = 16
NL = 2
RMAX = 5.0

N = 512
NBLK = EC // N

F32 = mybir.dt.float32
BF16 = mybir.dt.bfloat16
ACT = mybir.ActivationFunctionType


def _Qnp():
    Q = np.zeros((5, 3, 3))
    s = 1.0 / np.sqrt(2.0)
    Q[0, 0, 1] = Q[0, 1, 0] = s
    Q[1, 1, 2] = Q[1, 2, 1] = s
    Q[2] = np.diag([-1.0, -1.0, 2.0]) / np.sqrt(6.0)
    Q[3, 0, 2] = Q[3, 2, 0] = s
    Q[4] = np.diag([1.0, -1.0, 0.0]) * s
    return Q


_Q = _Qnp()
_An = np.einsum('mij,pjk,qki->mpq', _Q, _Q, _Q)
_A = 0.5 * (_An + _An.transpose(0, 2, 1))


def _fold_weights(inp):
    f = lambda a: np.ascontiguousarray(a, dtype=np.float32)
    W = {}
    s0 = 1.0 / np.sqrt(3.0 * C)
    s1 = 1.0 / np.sqrt(4.0 * C)
    s2 = 1.0 / np.sqrt(4.0 * C)

    We1 = np.asarray(inp["W_e1"], np.float64)
    be1 = np.asarray(inp["b_e1"], np.float64)
    We2 = np.asarray(inp["W_e2"], np.float64)
    be2 = np.asarray(inp["b_e2"], np.float64)
    Wenv_e = np.asarray(inp["W_env_e"], np.float64)
    benv_e = np.asarray(inp["b_env_e"], np.float64)
    Wenv = np.asarray(inp["Wenv"], np.float64)
    benv = np.asarray(inp["benv"], np.float64)
    WM1 = np.asarray(inp["Wm1"], np.float64)
    bM1 = np.asarray(inp["bm1"], np.float64)
    WM2 = np.asarray(inp["Wm2"], np.float64)
    bM2 = np.asarray(inp["bm2"], np.float64)
    wtp = np.asarray(inp["w_tp"], np.float64)
    WL0 = np.asarray(inp["Wlin0"], np.float64)
    WL1 = np.asarray(inp["Wlin1"], np.float64)
    WL2 = np.asarray(inp["Wlin2"], np.float64)

    import ml_dtypes as _mld
    _bias1 = float(np.float32(1.2784645427610783).astype(_mld.bfloat16))
    ones_val = _bias1 / (1.0 + np.exp(-_bias1))   # exact value of comb[64]/mh0[64]

    W["Wpe1"] = f(We1)
    W["be1"] = f(be1.reshape(S, 1))
    W["Wpe2"] = f(np.hstack([We2, np.zeros((S, 1))]))          # (64,65)
    W["be2"] = f(np.vstack([be2.reshape(S, 1), [[_bias1]]]))   # (65,1)

    def tile8(w):
        return np.concatenate([w] * 8, axis=1)

    W["Wenv128_0"] = f(np.vstack([
        tile8(Wenv_e), tile8(benv_e.reshape(1, C)) / ones_val]))
    W["Wenv128_1"] = f(np.vstack([
        tile8(WM2[0] @ Wenv[0]),
        tile8((bM2[0] @ Wenv[0] + benv[0]).reshape(1, C)) / ones_val,
    ]))

    w = wtp[0]
    W0, W1, W2 = WL0[0], WL1[0], WL2[0]

    def n1col(a):
        z = np.zeros((a.shape[0], 80)); z[:, 0:48] = a; return z

    def n0col(a):
        z = np.zeros((a.shape[0], 80)); z[:, 64:80] = a; return z

    w011 = np.zeros((48, 48))
    w101 = np.zeros((48, 48))
    for i in range(3):
        for c in range(C):
            w011[i * C + c, i * C:(i + 1) * C] = w[1][c] * W1[c] * s1
            w101[i * C + c, i * C:(i + 1) * C] = w[3][c] * W1[16 + c] * s1
    w01 = w011 + w101
    w1221 = []
    for j in range(3):
        wj = np.zeros((80, 48))
        for m in range(5):
            for i in range(3):
                for c in range(C):
                    wj[m * C + c, i * C:(i + 1) * C] += _Q[m, i, j] * (
                        w[6][c] * W1[32 + c] + w[8][c] * W1[48 + c]) * s1
        w1221.append(wj)
    w022 = np.zeros((80, 80))
    w202 = np.zeros((80, 80))
    for m in range(5):
        for c in range(C):
            w022[m * C + c, m * C:(m + 1) * C] = w[2][c] * W2[c] * s2
            w202[m * C + c, m * C:(m + 1) * C] = w[7][c] * W2[32 + c] * s2
    w02 = w022 + w202
    w112 = []
    for j in range(3):
        wj = np.zeros((48, 80))
        for i in range(3):
            for m in range(5):
                for c in range(C):
                    wj[i * C + c, m * C:(m + 1) * C] += (
                        _Q[m, i, j] * w[5][c] * W2[16 + c] * s2)
        w112.append(wj)
    w222 = []
    for q in range(5):
        wq = np.zeros((80, 80))
        for p in range(5):
            for m in range(5):
                for c in range(C):
                    wq[p * C + c, m * C:(m + 1) * C] += (
                        _A[m, p, q] * w[10][c] * W2[48 + c] * s2)
        w222.append(wq)

    def wt000(l):
        return (wtp[l][0][:, None] * WL0[l][0:16]) * s0

    def w110f(l):
        z = np.zeros((48, 16))
        for i in range(3):
            z[i * C:(i + 1) * C] = wtp[l][4][:, None] * WL0[l][16:32] * s0
        return z

    def w220f(l):
        z = np.zeros((80, 16))
        for m in range(5):
            z[m * C:(m + 1) * C] = wtp[l][9][:, None] * WL0[l][32:48] * s0
        return z

    # pn01 terms (out 80: n1 @0:48, n0 @64:80)
    W["Wx1_01"] = f(n1col(w01))                        # (48,80)  rhs x1t
    for j in range(3):
        W[f"WP3_01_{j}"] = f(n1col(w1221[j]))          # (80,80)  rhs TG80 g j
    W["WP4d_01"] = f(n0col(w220f(0)))                  # (80,80)  rhs TG80 g3
    W["WP1d_01"] = f(n0col(w110f(0)))                  # (48,80)  rhs TG48 g3
    W["WG01"] = f(np.vstack([
        n0col(Wenv_e @ wt000(0)),
        n0col((benv_e @ wt000(0)).reshape(1, 16)) / ones_val,
    ]))                                                # (65,80)  rhs comb0[0:65]

    # pn2 terms (out 80)
    W["Wx2_2"] = f(w02)                                # (80,80)  rhs x2t
    for j in range(3):
        W[f"WP1_2_{j}"] = f(w112[j])                   # (48,80)  rhs TG48 g j
    for q in range(5):
        W[f"WP4_2_{q}"] = f(w222[q])                   # (80,80)  rhs TG80 g 4+q

    W["Wm1s"] = f(np.hstack([WM1[0][0:64], np.zeros((S, 1))]))  # (64,65)
    W["bm1_0"] = f(np.vstack([bM1[0].reshape(S, 1), [[_bias1]]]))
    Wm1i = np.zeros((80, 65)); Wm1i[64:80, 0:64] = WM1[0][64:80]
    W["Wm1i"] = f(Wm1i)                                # (80,65)  rhs NDC
    W["Wm2"] = f(WM2[0])
    W["bm2_0"] = f(bM2[0].reshape(S, 1))

    W["WUBa"] = f(w220f(1))                            # (80,16)  rhs TUAt
    WUBX = np.zeros((80, 16))
    WUBX[0:48] = w110f(1)
    WUBX[64:80] = wt000(1)
    W["WUBX"] = f(WUBX)                                # (80,16)  rhs TUXt
    W["Wm1sb"] = f(WM1[1][0:64])
    W["bm1_1"] = f(bM1[1].reshape(S, 1))
    W["Wm1ib"] = f(WM1[1][64:80])                      # (16,64)  rhs n0bt
    W["Wm2b"] = f(WM2[1])
    W["bm2_1"] = f(bM2[1].reshape(S, 1))
    return W


def _pack_weights(W):
    names = list(W.keys())
    offs = {}
    col = 0
    for nm in names:
        k, m = W[nm].shape
        offs[nm] = (k, m, col)
        col += m
    arr = np.zeros((128, col), BF)
    for nm in names:
        k, m, o = offs[nm]
        arr[:k, o:o + m] = W[nm].astype(BF)
    return arr, offs


def _build_nc(woffs, wcols):
    nc = bacc.Bacc()
    h_p = nc.declare_dram_parameter("h", [24, EC], BF16, isOutput=False)
    g_p = nc.declare_dram_parameter("geom", [9, EC], BF16, isOutput=False)
    wpack_p = nc.declare_dram_parameter("wpack", [128, wcols], BF16, isOutput=False)
    out_p = nc.declare_dram_parameter("out", [NL, S, EC], BF16, isOutput=True)

    h_ap = h_p[:]
    g_ap = g_p[:]
    out_ap = out_p[:]

    def gsrc(offset, pattern):
        return bass.AP(tensor=g_ap.tensor, offset=offset, ap=pattern)

    def rep(tile_ap, k, n):
        return bass.AP(tensor=tile_ap.tensor, offset=tile_ap.offset,
                       ap=[[tile_ap.ap[0][0], tile_ap.ap[0][1]], [0, k], [1, n]])

    with TileContext(nc) as tc:
        with (
            tc.tile_pool(name="const", bufs=1) as constp,
            tc.tile_pool(name="work", bufs=4) as work,
            tc.tile_pool(name="psum", bufs=4, space="PSUM") as psumA,
            tc.tile_pool(name="psumw", bufs=4, space="PSUM") as psumB,
        ):
            wpack = constp.tile([128, wcols], BF16, name="wpack", tag="wpack")
            nc.sync.dma_start(out=wpack, in_=wpack_p[:])

            class _WT:
                def __getitem__(self, nm):
                    k, m, o = woffs[nm]
                    return wpack[:k, o:o + m]

            wt = _WT()

            def ps(nm, parts):
                if parts <= 64:
                    return psumA.tile([80, N], F32, name=nm, tag="psA")
                return psumB.tile([128, N], F32, name=nm, tag="psB")

            # PE warm-up burst once weights land
            warm = psumA.tile([80, N], F32, name="warm", tag="psA")
            nc.tensor.matmul(warm[:1, :1], wpack[:1, :1], wpack[:1, :1],
                             start=True, stop=True)
            for _ in range(10):
                nc.tensor.matmul(warm[:64, :N], wpack[:128, :64],
                                 wpack[:128, 256:256 + N], start=True, stop=True)

            def head(blk):
                o = blk * N
                hT = work.tile([24, N], BF16, name="hT", tag="hT")
                nc.sync.dma_start(out=hT, in_=h_ap[:, slice(o, o + N)])

                GEO80 = work.tile([80, 9 * N], BF16, name="GEO80", tag="GEO80")
                nc.gpsimd.dma_start(
                    out=GEO80[:, 0:3 * N],
                    in_=gsrc(5 * EC + o, [[0, 80], [EC, 3], [1, N]]))
                nc.gpsimd.dma_start(
                    out=GEO80[:, 3 * N:4 * N],
                    in_=gsrc(o, [[EC, 5], [0, 16], [1, N]]))
                nc.gpsimd.dma_start(
                    out=GEO80[:, 4 * N:9 * N],
                    in_=gsrc(o, [[0, 80], [EC, 5], [1, N]]))

                GEO48 = work.tile([48, 4 * N], BF16, name="GEO48", tag="GEO48")
                nc.sync.dma_start(
                    out=GEO48[:, 0:3 * N],
                    in_=gsrc(5 * EC + o, [[0, 48], [EC, 3], [1, N]]))
                nc.sync.dma_start(
                    out=GEO48[:, 3 * N:4 * N],
                    in_=gsrc(5 * EC + o, [[EC, 3], [0, 16], [1, N]]))

                GN = work.tile([80, N], BF16, name="GN", tag="GN")
                nc.gpsimd.dma_start(
                    out=GN[0:48], in_=gsrc(5 * EC + o, [[EC, 3], [0, 16], [1, N]]))
                nc.gpsimd.dma_start(
                    out=GN[48:80], in_=gsrc(8 * EC + o, [[0, 32], [1, N]]))

                pe1 = ps("pe1", 64)
                nc.tensor.matmul(pe1[:64], wt["Wpe1"], hT, start=True, stop=True)
                sb1 = work.tile([64, N], BF16, name="sb1", tag="sb1")
                nc.scalar.activation(sb1, pe1[:64], ACT.Silu, bias=wt["be1"])
                pe2 = ps("pe2", 64)
                nc.tensor.matmul(pe2[:65], wt["Wpe2"], sb1, start=True, stop=True)
                comb0 = work.tile([65, N], BF16, name="comb0", tag="comb0")
                nc.scalar.activation(comb0[0:65], pe2[:65], ACT.Silu, bias=wt["be2"])

                env0 = ps("env0", 128)
                nc.tensor.matmul(env0[:128], wt["Wenv128_0"], comb0[0:65],
                                 start=True, stop=True)

                gdiag = GEO80[:, 3 * N:4 * N]
                ddiag = GEO48[:, 3 * N:4 * N]
                x2t = work.tile([80, N], BF16, name="x2t", tag="x2t")
                nc.vector.tensor_mul(x2t, env0[0:80], gdiag)
                x1t = work.tile([48, N], BF16, name="x1t", tag="x1t")
                nc.vector.tensor_mul(x1t, env0[64:112], ddiag)
                TG80 = work.tile([80, 9 * N], BF16, name="TG80", tag="TG80")
                nc.vector.tensor_mul(TG80, rep(x2t[:], 9, N), GEO80)
                TG48 = work.tile([48, 4 * N], BF16, name="TG48", tag="TG48")
                nc.vector.tensor_mul(TG48, rep(x1t[:], 4, N), GEO48)

                pn01 = ps("pn01", 80)
                t01 = [("Wx1_01", x1t[:]), ("WP3_01_0", TG80[:, 0:N]),
                       ("WP3_01_1", TG80[:, N:2 * N]), ("WP3_01_2", TG80[:, 2 * N:3 * N]),
                       ("WP4d_01", TG80[:, 3 * N:4 * N]), ("WP1d_01", TG48[:, 3 * N:4 * N]),
                       ("WG01", comb0[0:65])]
                for i, (wn, rhs) in enumerate(t01):
                    nc.tensor.matmul(pn01[:80], wt[wn], rhs,
                                     start=(i == 0), stop=(i == len(t01) - 1))
                pn2 = ps("pn2", 80)
                t2 = [("Wx2_2", x2t[:]), ("WP1_2_0", TG48[:, 0:N]),
                      ("WP1_2_1", TG48[:, N:2 * N]), ("WP1_2_2", TG48[:, 2 * N:3 * N]),
                      ("WP4_2_0", TG80[:, 4 * N:5 * N]), ("WP4_2_1", TG80[:, 5 * N:6 * N]),
                      ("WP4_2_2", TG80[:, 6 * N:7 * N]), ("WP4_2_3", TG80[:, 7 * N:8 * N]),
                      ("WP4_2_4", TG80[:, 8 * N:9 * N])]
                for i, (wn, rhs) in enumerate(t2):
                    nc.tensor.matmul(pn2[:80], wt[wn], rhs,
                                     start=(i == 0), stop=(i == len(t2) - 1))
                NDC = work.tile([80, N], BF16, name="NDC", tag="NDC")
                nc.vector.tensor_mul(NDC, pn01[:80], GN)
                NG = work.tile([80, N], BF16, name="NG", tag="NG")
                nc.vector.tensor_mul(NG, pn2[:80], gdiag)
                return dict(comb0=comb0, NDC=NDC, NG=NG)

            def tail(blk, st):
                o = blk * N
                sl = slice(o, o + N)
                comb0, NDC, NG = st["comb0"], st["NDC"], st["NG"]

                pm1 = ps("pm1", 64)
                nc.tensor.matmul(pm1[:65], wt["Wm1s"], comb0[0:64],
                                 start=True, stop=False)
                nc.tensor.matmul(pm1[:65], wt["Wm1i"], NDC,
                                 start=False, stop=True)
                mh0 = work.tile([65, N], BF16, name="mh0", tag="mh0")
                nc.scalar.activation(mh0[0:65], pm1[:65], ACT.Silu, bias=wt["bm1_0"])
                pm2 = ps("pm2", 64)
                nc.tensor.matmul(pm2[:64], wt["Wm2"], mh0[0:64],
                                 start=True, stop=True)
                comb1 = work.tile([64, N], BF16, name="comb1", tag="comb1")
                nc.scalar.activation(comb1, pm2[:64], ACT.Identity, bias=wt["bm2_0"])
                nc.sync.dma_start(out=out_ap[0, :, sl], in_=comb1)

                env1 = ps("env1", 128)
                nc.tensor.matmul(env1[:128], wt["Wenv128_1"], mh0[0:65],
                                 start=True, stop=True)

                TUAt = work.tile([80, N], BF16, name="TUAt", tag="TUAt")
                nc.vector.tensor_mul(TUAt, NG, env1[0:80])
                TUXt = work.tile([80, N], BF16, name="TUXt", tag="TUXt")
                nc.vector.tensor_mul(TUXt, NDC, env1[0:80])

                pn0b = ps("pn0b", 16)
                nc.tensor.matmul(pn0b[:16], wt["WUBa"], TUAt, start=True, stop=False)
                nc.tensor.matmul(pn0b[:16], wt["WUBX"], TUXt, start=False, stop=True)
                n0bt = work.tile([16, N], BF16, name="n0bt", tag="n0bt")
                nc.scalar.activation(n0bt, pn0b[:16], ACT.Copy)

                pm1b = ps("pm1b", 64)
                nc.tensor.matmul(pm1b[:64], wt["Wm1sb"], comb1,
                                 start=True, stop=False)
                nc.tensor.matmul(pm1b[:64], wt["Wm1ib"], n0bt,
                                 start=False, stop=True)
                mh1 = work.tile([64, N], BF16, name="mh1", tag="mh1")
                nc.scalar.activation(mh1, pm1b[:64], ACT.Silu, bias=wt["bm1_1"])
                pm2b = ps("pm2b", 64)
                nc.tensor.matmul(pm2b[:64], wt["Wm2b"], mh1, start=True, stop=True)
                scal2 = work.tile([64, N], BF16, name="scal2", tag="scal2")
                nc.scalar.activation(scal2, pm2b[:64], ACT.Identity, bias=wt["bm2_1"])
                nc.sync.dma_start(out=out_ap[1, :, sl], in_=scal2)

            sts = [head(0), head(1)]
            warmb = psumA.tile([80, N], F32, name="warmb", tag="psA")
            for _ in range(8):
                nc.tensor.matmul(warmb[:64, :N], wpack[:128, :64],
                                 wpack[:128, 256:256 + N], start=True, stop=True)
            for blk in range(NBLK):
                if blk + 2 < NBLK:
                    sts.append(head(blk + 2))
                tail(blk, sts[blk])
                sts[blk] = None
    nc.finalize()
    return nc


_NC_CACHE = None


def _host_prep(inputs):
    bond_dist = np.asarray(inputs["bond_dist"], np.float32)
    bond_diff = np.asarray(inputs["bond_diff"], np.float32)
    emb = np.asarray(inputs["emb_table"], np.float32)
    Z = np.asarray(inputs["Z"]).astype(np.int64)
    ei = np.asarray(inputs["edge_index"]).astype(np.int64)

    u = bond_dist / RMAX
    n = np.arange(1, NB + 1, dtype=np.float32)
    radial = (np.sqrt(np.float32(2.0 / RMAX)) *
              np.sin(np.float32(np.pi) * n * u[:, None].astype(np.float32)) /
              bond_dist[:, None])
    cutoff = np.where(u < 1.0, 1.0 - 28.0 * u**6 + 48.0 * u**7 - 21.0 * u**8, 0.0)
    radial = (radial * cutoff[:, None].astype(np.float32)).astype(np.float32)

    d = (bond_diff / (bond_dist[:, None] + np.float32(1e-8))).astype(np.float32)
    y2 = (np.sqrt(np.float32(1.5)) *
          np.einsum('mij,ei,ej->em', _Q.astype(np.float32), d, d)).astype(np.float32)

    te = (emb[Z[ei[:, 0]]] * emb[Z[ei[:, 1]]]).astype(np.float32)

    h = np.ascontiguousarray(np.concatenate([radial, te], axis=1).T.astype(BF))
    ones = np.ones((E, 1), np.float32)
    geom = np.ascontiguousarray(
        np.concatenate([y2, d, ones], axis=1).T.astype(BF))
    W = _fold_weights(inputs)
    return h, geom, W


def make_in_maps(inputs):
    global _NC_CACHE
    h, geom, W = _host_prep(inputs)
    wpack, woffs = _pack_weights(W)
    if _NC_CACHE is None:
        _NC_CACHE = _build_nc(woffs, wpack.shape[1])
    in_maps = []
    for i in range(NCORES):
        sl = slice(i * EC, (i + 1) * EC)
        m = {"h": np.ascontiguousarray(h[:, sl]),
             "geom": np.ascontiguousarray(geom[:, sl]),
             "wpack": wpack}
        in_maps.append(m)
    return in_maps


def kernel(**inputs):
    in_maps = make_in_maps(inputs)
    res = run_bass_kernel_spmd(_NC_CACHE, in_maps, list(range(NCORES))).results
    out = np.concatenate(
        [np.asarray(res[i]["out"]).astype(np.float32).transpose(2, 0, 1)
         for i in range(NCORES)], axis=0)
    return np.ascontiguousarray(out)
